# revision 7
# baseline (speedup 1.0000x reference)
"""Trainium2 Bass kernel for nn_EntityPredictor (bidirectional span LSTM entity scorer).

Strategy (8 NeuronCores, data-parallel over spans):
  - Host: sort spans by length desc, pad with dummy spans so every core gets an
    identical length histogram (n=spans/core, schedule n_t = #spans with len>t).
    Spans are dealt round-robin to the 8 cores, so one compiled SPMD program works
    for all cores.
  - Device (per core), "transposed" layout (gate/hidden dim on partitions, spans on
    the free dim):
      1. Indirect-DMA gather of the valid tokens (t-major staircase) from the full
         hidden_layers table resident in DRAM.
      2. PE-transpose the gathered [tok, 768] tiles into xT [768, tok].
      3. XG = W_ih'^T.T @ xT for both directions, 8 gate chunks x 1 PSUM bank each,
         kept resident in PSUM.
      4. 5 recurrence steps per direction: PE accumulates W_hh'^T.T @ hT on top of
         the XG slice for that timestep (start=False), ACT applies
         sigmoid/tanh with the per-gate bias fused, DVE computes c/h updates and the
         masked time-sum of h. Forward walks t ascending; backward walks t
         DESCENDING, which makes x_rev reversal completely free (each span consumes
         token t at global step t in both directions).
      5. logits^T [16, n] = E'^T.T @ [hsum_f; hsum_b] and DMA out.
  - Host: unpermute spans, drop dummies, return [1280, 16] float32.

Gate padding: each gate block (200) is padded to 256 so gate boundaries align with
128-partition chunks; gate order is permuted to (i, f, o, g) so chunks 0-5 are all
sigmoid and chunks 6-7 are tanh. Padded rows produce h=0 and contribute nothing.
"""

import numpy as np

BSZ, SEQ, D, H, L = 64, 512, 768, 200, 5
N_ENT = 16
NCORES = 8
HP = 256          # padded gate block
G4 = 4 * HP       # 1024 padded gate rows
KT = [(0, 128), (128, 72)]  # hidden-dim k-tiles (200 rows)

_CACHE = {}


# ---------------------------------------------------------------- host planning

def _plan(span_len, span_batch, span_token_idx):
    lens = np.asarray(span_len).astype(np.int64)
    NS = lens.shape[0]
    flat = (np.asarray(span_batch).astype(np.int64)[:, None] * SEQ
            + np.asarray(span_token_idx).astype(np.int64))       # [NS, L]
    hist = np.bincount(lens, minlength=L + 1)[1:]
    pad = (-hist) % NCORES
    all_len = np.concatenate([lens, np.repeat(np.arange(1, L + 1), pad)])
    all_flat = np.concatenate([flat, np.zeros((int(pad.sum()), L), np.int64)])
    order = np.argsort(-all_len, kind="stable")                  # length desc
    N = all_len.shape[0]
    n = N // NCORES
    cores = [order[k::NCORES] for k in range(NCORES)]            # [n] ids, len desc
    n_t = tuple(int((all_len[cores[0]] > t).sum()) for t in range(L))
    for k in range(1, NCORES):
        assert tuple(int((all_len[cores[k]] > t).sum()) for t in range(L)) == n_t
    offs = np.concatenate([[0], np.cumsum(n_t)]).astype(int)
    ntok = int(offs[-1])
    nchunk = (ntok + 127) // 128
    ntok_pad = nchunk * 128
    gidx = np.zeros((NCORES, 128, nchunk), np.int32)
    for k in range(NCORES):
        lst = np.concatenate([all_flat[cores[k][: n_t[t]], t] for t in range(L)])
        buf = np.zeros(ntok_pad, np.int64)
        buf[:ntok] = lst
        gidx[k] = buf.reshape(nchunk, 128).T.astype(np.int32)
    return dict(cores=cores, n=n, n_t=n_t, offs=offs, ntok=ntok,
                nchunk=nchunk, ntok_pad=ntok_pad, NS=NS)


def _prep_dir(W_ih, W_hh, b_ih, b_hh):
    """Pad gates 200->256, permute gate order (i,f,g,o)->(i,f,o,g), transpose."""
    W_ih = np.asarray(W_ih, np.float32)
    W_hh = np.asarray(W_hh, np.float32)
    b = np.asarray(b_ih, np.float32) + np.asarray(b_hh, np.float32)
    A = np.zeros((G4, D), np.float32)
    B = np.zeros((G4, H), np.float32)
    bias = np.zeros((G4,), np.float32)
    for newg, oldg in enumerate([0, 1, 3, 2]):                   # i, f, o, g
        dst = slice(newg * HP, newg * HP + H)
        src = slice(oldg * H, (oldg + 1) * H)
        A[dst] = W_ih[src]
        B[dst] = W_hh[src]
        bias[dst] = b[src]
    # [768, 1024], [200, 1024], bias as [128, 8] (col = gate chunk)
    return A.T.copy(), B.T.copy(), bias.reshape(8, 128).T.copy()


# ---------------------------------------------------------------- device program

def _build(meta):
    import concourse.bacc as bacc
    import concourse.bass as bass
    import concourse.mybir as mybir
    import concourse.tile as tile
    from concourse.masks import make_identity

    f32 = mybir.dt.float32
    i32 = mybir.dt.int32
    AF = mybir.ActivationFunctionType
    n, n_t, offs, ntok = meta["n"], meta["n_t"], meta["offs"], meta["ntok"]
    nchunk, ntok_pad = meta["nchunk"], meta["ntok_pad"]

    nc = bacc.Bacc("TRN2", target_bir_lowering=False, debug=False,
                   num_devices=NCORES)
    table = nc.dram_tensor("table", [BSZ * SEQ, D], f32, kind="ExternalInput").ap()
    gidx_d = nc.dram_tensor("gidx", [128, nchunk], i32, kind="ExternalInput").ap()
    A_d = {dd: nc.dram_tensor(f"A_{dd}", [D, G4], f32, kind="ExternalInput").ap()
           for dd in "fb"}
    B_d = {dd: nc.dram_tensor(f"B_{dd}", [H, G4], f32, kind="ExternalInput").ap()
           for dd in "fb"}
    bias_d = nc.dram_tensor("bias", [128, 16], f32, kind="ExternalInput").ap()
    et_d = nc.dram_tensor("ET", [2 * H, N_ENT], f32, kind="ExternalInput").ap()
    out_d = nc.dram_tensor("out", [N_ENT, n], f32, kind="ExternalOutput").ap()

    with tile.TileContext(nc) as tc:
        with tc.tile_pool(name="const", bufs=1) as const, \
             tc.tile_pool(name="gx", bufs=3) as gxp, \
             tc.tile_pool(name="state", bufs=1) as statep, \
             tc.tile_pool(name="work", bufs=2) as workp:

            ident = const.tile([128, 128], f32)
            make_identity(nc, ident[:])
            idx_sb = const.tile([128, nchunk], i32)
            nc.sync.dma_start(out=idx_sb[:], in_=gidx_d[:, :])
            bias_sb = const.tile([128, 16], f32)
            nc.sync.dma_start(out=bias_sb[:], in_=bias_d[:, :])
            et_tiles = []
            for p0, pl in [(0, 128), (128, 72), (200, 128), (328, 72)]:
                tt = const.tile([pl, N_ENT], f32, tag=f"et{p0}")
                nc.sync.dma_start(out=tt[:], in_=et_d[p0:p0 + pl, :])
                et_tiles.append(tt)
            A_sb, B_sb = {}, {}
            for dd in "fb":
                A_sb[dd] = []
                for k in range(6):
                    t_ = const.tile([128, G4], f32, tag=f"A{dd}{k}")
                    nc.sync.dma_start(out=t_[:], in_=A_d[dd][k * 128:(k + 1) * 128, :])
                    A_sb[dd].append(t_)
                B_sb[dd] = []
                for j, (p0, pl) in enumerate(KT):
                    t_ = const.tile([pl, G4], f32, tag=f"B{dd}{j}")
                    nc.sync.dma_start(out=t_[:], in_=B_d[dd][p0:p0 + pl, :])
                    B_sb[dd].append(t_)

            # ---- gather + transpose -> xT[k] = x^T [768, ntok_pad]
            xT = [const.tile([128, ntok_pad], f32, tag=f"xT{k}", name=f"xT{k}")
                  for k in range(6)]
            with tc.tile_pool(name="tp", bufs=2, space="PSUM") as tpp:
                for c in range(nchunk):
                    gx = gxp.tile([128, D], f32)
                    nc.gpsimd.indirect_dma_start(
                        out=gx[:], out_offset=None, in_=table[:, :],
                        in_offset=bass.IndirectOffsetOnAxis(
                            ap=idx_sb[:, c:c + 1], axis=0))
                    for k in range(6):
                        pt = tpp.tile([128, 128], f32)
                        nc.tensor.transpose(pt[:], gx[:, k * 128:(k + 1) * 128],
                                            ident[:])
                        nc.vector.tensor_copy(
                            out=xT[k][:, c * 128:(c + 1) * 128], in_=pt[:])

            # ---- per direction: XG -> PSUM -> SBUF (bias fused), then recurrence
            hsum = {}
            xg_sb = [const.tile([128, ntok], f32, tag=f"xgsb{m}", name=f"xgsb{m}")
                     for m in range(8)]
            for di, dd in enumerate("fb"):
                with tc.tile_pool(name=f"xg{dd}", bufs=1, space="PSUM") as xgp:
                    xg = [xgp.tile([128, ntok], f32, tag=f"xg{m}", name=f"xg{dd}{m}")
                          for m in range(8)]
                    for m in range(8):
                        for k in range(6):
                            nc.tensor.matmul(
                                xg[m][:, :],
                                A_sb[dd][k][:, m * 128:(m + 1) * 128],
                                xT[k][:, :ntok],
                                start=(k == 0), stop=(k == 5))
                    # xg_sb = xg + bias (per-gate bias fused into the copy)
                    for m in range(8):
                        nc.scalar.activation(
                            xg_sb[m][:, :], xg[m][:, :], AF.Identity,
                            bias=bias_sb[:, di * 8 + m: di * 8 + m + 1])
                with tc.tile_pool(name=f"pre{dd}", bufs=1, space="PSUM") as prep:
                    hT = [statep.tile([pl, n], f32, tag=f"hT{j}", name=f"hT{dd}{j}")
                          for j, (p0, pl) in enumerate(KT)]
                    cst = [statep.tile([pl, n], f32, tag=f"c{j}", name=f"c{dd}{j}")
                           for j, (p0, pl) in enumerate(KT)]
                    hs = [statep.tile([pl, n], f32, tag=f"hs{dd}{j}", name=f"hs{dd}{j}")
                          for j, (p0, pl) in enumerate(KT)]
                    for t_ in hT + cst + hs:
                        nc.vector.memset(t_[:], 0.0)
                    steps = list(range(L)) if dd == "f" else list(range(L - 1, -1, -1))
                    for si, t in enumerate(steps):
                        w = n_t[t]
                        o0 = int(offs[t])
                        pre = [prep.tile([128, w], f32, tag=f"pre{m}",
                                         name=f"pre{m}_{dd}{t}")
                               for m in range(8)]
                        for m in range(8):
                            nc.tensor.matmul(
                                pre[m][:, :], ident[:],
                                xg_sb[m][:, o0:o0 + w],
                                start=True, stop=False)
                            for j, (p0, pl) in enumerate(KT):
                                nc.tensor.matmul(
                                    pre[m][:, :],
                                    B_sb[dd][j][:, m * 128:(m + 1) * 128],
                                    hT[j][:, :w],
                                    start=False, stop=(j == 1))
                        gact = [workp.tile([128, n], f32, tag=f"gact{m}", name=f"gact{m}_{dd}{t}")
                                for m in range(8)]
                        for m in range(8):
                            nc.scalar.activation(
                                gact[m][:, :w], pre[m][:, :],
                                AF.Sigmoid if m < 6 else AF.Tanh)
                        tmp = [workp.tile([pl, n], f32, tag=f"tmp{j}", name=f"tmp{j}_{dd}{t}")
                               for j, (p0, pl) in enumerate(KT)]
                        tnc = [workp.tile([pl, n], f32, tag=f"tnc{j}", name=f"tnc{j}_{dd}{t}")
                               for j, (p0, pl) in enumerate(KT)]
                        for j, (p0, pl) in enumerate(KT):
                            ii = gact[0 + j][:pl, :w]
                            ff = gact[2 + j][:pl, :w]
                            oo = gact[4 + j][:pl, :w]
                            gg = gact[6 + j][:pl, :w]
                            nc.vector.tensor_mul(tmp[j][:, :w], ii, gg)
                            nc.vector.tensor_mul(cst[j][:, :w], cst[j][:, :w], ff)
                            nc.vector.tensor_add(cst[j][:, :w], cst[j][:, :w],
                                                 tmp[j][:, :w])
                            nc.scalar.activation(tnc[j][:, :w], cst[j][:, :w],
                                                 AF.Tanh)
                            nc.vector.tensor_mul(hT[j][:, :w], oo, tnc[j][:, :w])
                            nc.vector.tensor_add(hs[j][:, :w], hs[j][:, :w],
                                                 hT[j][:, :w])
                    hsum[dd] = hs

            # ---- logits^T = E'^T.T @ [hsum_f; hsum_b]
            with tc.tile_pool(name="lg", bufs=1, space="PSUM") as lgp:
                lg = lgp.tile([N_ENT, n], f32)
                rhs_tiles = hsum["f"] + hsum["b"]
                for j4 in range(4):
                    nc.tensor.matmul(lg[:, :], et_tiles[j4][:], rhs_tiles[j4][:],
                                     start=(j4 == 0), stop=(j4 == 3))
                out_sb = const.tile([N_ENT, n], f32)
                nc.vector.tensor_copy(out=out_sb[:], in_=lg[:, :])
                nc.sync.dma_start(out=out_d[:, :], in_=out_sb[:])

    nc.compile()
    return nc


# ---------------------------------------------------------------- entry points

def run(inputs, trace=False, trace_cores=None):
    from concourse.bass_utils import run_bass_kernel_spmd

    meta = _plan(inputs["span_len"], inputs["span_batch"],
                 inputs["span_token_idx"])
    key = (meta["n"], meta["n_t"], meta["ntok"])
    if key not in _CACHE:
        _CACHE[key] = _build(meta)
    nc = _CACHE[key]

    A_f, B_f, bias_f = _prep_dir(inputs["W_ih_f"], inputs["W_hh_f"],
                                 inputs["b_ih_f"], inputs["b_hh_f"])
    A_b, B_b, bias_b = _prep_dir(inputs["W_ih_b"], inputs["W_hh_b"],
                                 inputs["b_ih_b"], inputs["b_hh_b"])
    bias = np.concatenate([bias_f, bias_b], axis=1)
    ET = np.asarray(inputs["entity_embs"], np.float32).T.copy()
    table = np.ascontiguousarray(
        np.asarray(inputs["hidden_layers"], np.float32).reshape(BSZ * SEQ, D))
    gidx_all = _gidx(inputs, meta)

    in_maps = [dict(table=table, gidx=gidx_all[k], A_f=A_f, A_b=A_b,
                    B_f=B_f, B_b=B_b, bias=bias, ET=ET)
               for k in range(NCORES)]
    res = run_bass_kernel_spmd(nc, in_maps, list(range(NCORES)),
                               trace=trace, trace_cores=trace_cores)
    n, NS = meta["n"], meta["NS"]
    logits = np.zeros((NS, N_ENT), np.float32)
    for k in range(NCORES):
        outk = res.results[k]["out"]                              # [16, n]
        ids = meta["cores"][k]
        sel = ids < NS
        logits[ids[sel]] = outk[:, sel].T
    return logits, res


def _gidx(inputs, meta):
    lens = np.asarray(inputs["span_len"]).astype(np.int64)
    flat = (np.asarray(inputs["span_batch"]).astype(np.int64)[:, None] * SEQ
            + np.asarray(inputs["span_token_idx"]).astype(np.int64))
    hist = np.bincount(lens, minlength=L + 1)[1:]
    pad = (-hist) % NCORES
    all_flat = np.concatenate([flat, np.zeros((int(pad.sum()), L), np.int64)])
    n_t, ntok_pad, nchunk = meta["n_t"], meta["ntok_pad"], meta["nchunk"]
    out = np.zeros((NCORES, 128, nchunk), np.int32)
    for k in range(NCORES):
        ids = meta["cores"][k]
        lst = np.concatenate([all_flat[ids[: n_t[t]], t] for t in range(L)])
        buf = np.zeros(ntok_pad, np.int64)
        buf[: meta["ntok"]] = lst
        out[k] = buf.reshape(nchunk, 128).T.astype(np.int32)
    return out


def kernel(**inputs):
    logits, _ = run(inputs, trace=False)
    return logits


# revision 9
# speedup vs baseline: 2.1350x; 2.1350x over previous
"""Trainium2 Bass kernel for nn_EntityPredictor (bidirectional span LSTM entity scorer).

Strategy (8 NeuronCores, data-parallel over spans):
  - Host: sort spans by length desc, pad with dummy spans so every core gets an
    identical length histogram (n=spans/core, schedule n_t = #spans with len>t,
    all even).  Spans are dealt round-robin to the 8 cores, so one compiled SPMD
    program serves all cores.
  - Device (per core), "transposed" layout (gate/hidden dim on partitions, spans
    on the free dim):
      1. Indirect-DMA gather of the valid tokens (t-major staircase) from the
         full hidden_layers table resident in DRAM.
      2. PE-transpose the gathered [tok, 768] tiles into xT [768, tok] (bf16).
      3. XG = W_ih'^T.T @ xT for both directions (bf16 matmuls into PSUM), then
         ACT-Identity copies PSUM -> SBUF bf16 with the per-gate bias fused.
      4. Recurrence, both directions interleaved step-by-step so PE/ACT/DVE
         pipeline across directions.  Per step and direction: PE computes
         identity-add of the XG slice + W_hh'^T.T @ h into a packed 3-bank PSUM
         tile; one merged ACT applies sigmoid to the i/f/o chunks and one the
         tanh to g; DVE updates c (fp32), h (bf16 slab, directly in matmul-rhs
         layout) and the masked time-sum of h (fp32).  Forward walks t
         ascending; backward walks t DESCENDING, which makes the x_rev reversal
         free (every span consumes token t at global step t in both directions).
      5. logits^T [16, n] = E'^T.T @ [hsum_f; hsum_b] (fp32) and DMA out.
  - Host: unpermute spans, drop dummies, return [1280, 16] float32.

Gate padding: each gate block (200) is padded to 256 so gate boundaries align
with 128-partition chunks; gate order is permuted to (i, f, o, g) so chunks 0-5
are sigmoid and chunks 6-7 tanh.  Padded rows produce h=0 and contribute nothing.
"""

import numpy as np

BSZ, SEQ, D, H, L = 64, 512, 768, 200, 5
N_ENT = 16
NCORES = 8
HP = 256          # padded gate block
G4 = 4 * HP       # 1024 padded gate rows
KT = [(0, 128), (128, 72)]  # hidden-dim k-tiles (200 rows)

_CACHE = {}


# ---------------------------------------------------------------- host planning

def _plan(span_len, span_batch, span_token_idx):
    lens = np.asarray(span_len).astype(np.int64)
    NS = lens.shape[0]
    flat = (np.asarray(span_batch).astype(np.int64)[:, None] * SEQ
            + np.asarray(span_token_idx).astype(np.int64))       # [NS, L]
    hist = np.bincount(lens, minlength=L + 1)[1:]
    # pad classes to multiples of 16 -> per-core class counts even -> even n_t
    pad = (-hist) % (2 * NCORES)
    all_len = np.concatenate([lens, np.repeat(np.arange(1, L + 1), pad)])
    order = np.argsort(-all_len, kind="stable")                  # length desc
    N = all_len.shape[0]
    n = N // NCORES
    cores = [order[k::NCORES] for k in range(NCORES)]            # [n] ids, len desc
    n_t = tuple(int((all_len[cores[0]] > t).sum()) for t in range(L))
    for k in range(1, NCORES):
        assert tuple(int((all_len[cores[k]] > t).sum()) for t in range(L)) == n_t
    assert all(w % 2 == 0 for w in n_t) and n % 2 == 0
    offs = np.concatenate([[0], np.cumsum(n_t)]).astype(int)
    ntok = int(offs[-1])
    nchunk = (ntok + 127) // 128
    ntok_pad = nchunk * 128
    return dict(cores=cores, n=n, n_t=n_t, offs=offs, ntok=ntok,
                nchunk=nchunk, ntok_pad=ntok_pad, NS=NS,
                n_pad_spans=int(pad.sum()))


def _gidx(inputs, meta):
    flat = (np.asarray(inputs["span_batch"]).astype(np.int64)[:, None] * SEQ
            + np.asarray(inputs["span_token_idx"]).astype(np.int64))
    all_flat = np.concatenate(
        [flat, np.zeros((meta["n_pad_spans"], L), np.int64)])
    n_t, ntok_pad, nchunk = meta["n_t"], meta["ntok_pad"], meta["nchunk"]
    out = np.zeros((NCORES, 128, nchunk), np.int32)
    for k in range(NCORES):
        ids = meta["cores"][k]
        lst = np.concatenate([all_flat[ids[: n_t[t]], t] for t in range(L)])
        buf = np.zeros(ntok_pad, np.int64)
        buf[: meta["ntok"]] = lst
        out[k] = buf.reshape(nchunk, 128).T.astype(np.int32)
    return out


def _prep_dir(W_ih, W_hh, b_ih, b_hh):
    """Pad gates 200->256, permute gate order (i,f,g,o)->(i,f,o,g), transpose."""
    import ml_dtypes
    W_ih = np.asarray(W_ih, np.float32)
    W_hh = np.asarray(W_hh, np.float32)
    b = np.asarray(b_ih, np.float32) + np.asarray(b_hh, np.float32)
    A = np.zeros((G4, D), np.float32)
    B = np.zeros((G4, H), np.float32)
    bias = np.zeros((G4,), np.float32)
    for newg, oldg in enumerate([0, 1, 3, 2]):                   # i, f, o, g
        dst = slice(newg * HP, newg * HP + H)
        src = slice(oldg * H, (oldg + 1) * H)
        A[dst] = W_ih[src]
        B[dst] = W_hh[src]
        bias[dst] = b[src]
    # A^T [768, 1024] bf16, B^T [200, 1024] bf16, bias [128, 8] f32
    return (A.T.astype(ml_dtypes.bfloat16).copy(),
            B.T.astype(ml_dtypes.bfloat16).copy(),
            bias.reshape(8, 128).T.copy())


# ---------------------------------------------------------------- device program

def _build(meta):
    import concourse.bacc as bacc
    import concourse.bass as bass
    import concourse.mybir as mybir
    import concourse.tile as tile
    from concourse.masks import make_identity

    f32 = mybir.dt.float32
    bf16 = mybir.dt.bfloat16
    i32 = mybir.dt.int32
    AF = mybir.ActivationFunctionType
    n, n_t, offs, ntok = meta["n"], meta["n_t"], meta["offs"], meta["ntok"]
    nchunk, ntok_pad = meta["nchunk"], meta["ntok_pad"]

    nc = bacc.Bacc("TRN2", target_bir_lowering=False, debug=False,
                   num_devices=NCORES)
    table = nc.dram_tensor("table", [BSZ * SEQ, D], f32, kind="ExternalInput").ap()
    gidx_d = nc.dram_tensor("gidx", [128, nchunk], i32, kind="ExternalInput").ap()
    A_d = {dd: nc.dram_tensor(f"A_{dd}", [D, G4], bf16, kind="ExternalInput").ap()
           for dd in "fb"}
    B_d = {dd: nc.dram_tensor(f"B_{dd}", [H, G4], bf16, kind="ExternalInput").ap()
           for dd in "fb"}
    bias_d = nc.dram_tensor("bias", [128, 16], f32, kind="ExternalInput").ap()
    et_d = nc.dram_tensor("ET", [2 * H, N_ENT], f32, kind="ExternalInput").ap()
    out_d = nc.dram_tensor("out", [N_ENT, n], f32, kind="ExternalOutput").ap()

    with tile.TileContext(nc) as tc:
        with tc.tile_pool(name="const", bufs=1) as const, \
             tc.tile_pool(name="gx", bufs=3) as gxp, \
             tc.tile_pool(name="state", bufs=1) as statep, \
             tc.tile_pool(name="work", bufs=2) as workp:

            ident = const.tile([128, 128], f32)
            make_identity(nc, ident[:])
            identb = const.tile([128, 128], bf16)
            nc.vector.tensor_copy(out=identb[:], in_=ident[:])
            idx_sb = const.tile([128, nchunk], i32)
            nc.sync.dma_start(out=idx_sb[:], in_=gidx_d[:, :])
            bias_sb = const.tile([128, 16], f32)
            nc.sync.dma_start(out=bias_sb[:], in_=bias_d[:, :])
            et_tiles = []
            for p0, pl in [(0, 128), (128, 72), (200, 128), (328, 72)]:
                tt = const.tile([pl, N_ENT], f32, tag=f"et{p0}")
                nc.sync.dma_start(out=tt[:], in_=et_d[p0:p0 + pl, :])
                et_tiles.append(tt)
            A_sb, B_sb = {}, {}
            for dd in "fb":
                A_sb[dd] = []
                for k in range(6):
                    t_ = const.tile([128, G4], bf16, tag=f"A{dd}{k}")
                    nc.sync.dma_start(out=t_[:], in_=A_d[dd][k * 128:(k + 1) * 128, :])
                    A_sb[dd].append(t_)
                B_sb[dd] = []
                for j, (p0, pl) in enumerate(KT):
                    t_ = const.tile([pl, G4], bf16, tag=f"B{dd}{j}")
                    nc.sync.dma_start(out=t_[:], in_=B_d[dd][p0:p0 + pl, :])
                    B_sb[dd].append(t_)

            # ---- gather + transpose -> xT[k] = x^T [768, ntok_pad] bf16
            xT = [const.tile([128, ntok_pad], bf16, tag=f"xT{k}", name=f"xT{k}")
                  for k in range(6)]
            with tc.tile_pool(name="tp", bufs=2, space="PSUM") as tpp:
                for c in range(nchunk):
                    gx = gxp.tile([128, D], f32)
                    nc.gpsimd.indirect_dma_start(
                        out=gx[:], out_offset=None, in_=table[:, :],
                        in_offset=bass.IndirectOffsetOnAxis(
                            ap=idx_sb[:, c:c + 1], axis=0))
                    for k in range(6):
                        pt = tpp.tile([128, 128], f32)
                        nc.tensor.transpose(pt[:], gx[:, k * 128:(k + 1) * 128],
                                            ident[:])
                        nc.vector.tensor_copy(
                            out=xT[k][:, c * 128:(c + 1) * 128], in_=pt[:])

            # ---- XG for both directions -> SBUF bf16 (bias fused in the copy)
            xg_sb = {dd: [const.tile([128, ntok], bf16, tag=f"xgsb{dd}{m}",
                                     name=f"xgsb{dd}{m}")
                          for m in range(8)] for dd in "fb"}
            for di, dd in enumerate("fb"):
                with tc.tile_pool(name=f"xg{dd}", bufs=1, space="PSUM") as xgp:
                    xg = [xgp.tile([128, ntok], f32, tag=f"xg{m}", name=f"xg{dd}{m}")
                          for m in range(8)]
                    for m in range(8):
                        for k in range(6):
                            nc.tensor.matmul(
                                xg[m][:, :],
                                A_sb[dd][k][:, m * 128:(m + 1) * 128],
                                xT[k][:, :ntok],
                                start=(k == 0), stop=(k == 5))
                    for m in range(8):
                        nc.scalar.activation(
                            xg_sb[dd][m][:, :], xg[m][:, :], AF.Identity,
                            bias=bias_sb[:, di * 8 + m: di * 8 + m + 1])

            # ---- interleaved recurrences (packed 3-bank PSUM per direction)
            hsl = {}   # h slab  [128, 2n] bf16   (block j at cols [j*n, j*n+w))
            csl = {}   # c slab  [128, 2n] f32
            hss = {}   # hsum    [128, 2n] f32
            for dd in "fb":
                hsl[dd] = statep.tile([128, 2 * n], bf16, name=f"hsl{dd}")
                csl[dd] = statep.tile([128, 2 * n], f32, name=f"csl{dd}")
                hss[dd] = statep.tile([128, 2 * n], f32, name=f"hss{dd}")
                nc.vector.memset(hsl[dd][:], 0.0)
                nc.vector.memset(csl[dd][:], 0.0)
                nc.vector.memset(hss[dd][:], 0.0)

            def blk2(tile_ap, w):
                # [128, 2, w] strided view of a [128, 2n] slab (blocks at 0, n)
                return tile_ap.rearrange("p (b q) -> p b q", b=2)[:, :, :w]

            with tc.tile_pool(name="pre", bufs=1, space="PSUM") as prep:
                step_order = {"f": list(range(L)), "b": list(range(L - 1, -1, -1))}
                for si in range(L):
                    for dd in "fb":
                        t = step_order[dd][si]
                        w = n_t[t]
                        o0 = int(offs[t])
                        pre = prep.tile([128, 3 * 512], f32, tag=f"pre{dd}",
                                        name=f"pre{dd}{t}")

                        def chunk(m, w=w, pre=pre):
                            off = (m // 3) * 512 + (m % 3) * w
                            return pre[:, off:off + w]

                        # identity-add of XG slice opens each bank's group
                        for m in range(8):
                            nc.tensor.matmul(
                                chunk(m), identb[:],
                                xg_sb[dd][m][:, o0:o0 + w],
                                start=(m % 3 == 0), stop=False)
                        for j, (p0, pl) in enumerate(KT):
                            for m in range(8):
                                nc.tensor.matmul(
                                    chunk(m),
                                    B_sb[dd][j][:, m * 128:(m + 1) * 128],
                                    hsl[dd][:pl, j * n:j * n + w],
                                    start=False,
                                    stop=(j == 1 and (m % 3 == 2 or m == 7)))
                        # merged activations: chunks 0-5 sigmoid, 6-7 tanh
                        gsig = workp.tile([128, 6 * w], bf16, tag="gsig",
                                          name=f"gsig{dd}{t}")
                        gtan = workp.tile([128, 2 * w], bf16, tag="gtan",
                                          name=f"gtan{dd}{t}")
                        # banks 0/1 each hold 3 w-packed sigmoid chunks
                        nc.scalar.activation(
                            gsig[:, 0:3 * w], pre[:, 0:3 * w], AF.Sigmoid)
                        nc.scalar.activation(
                            gsig[:, 3 * w:6 * w], pre[:, 512:512 + 3 * w],
                            AF.Sigmoid)
                        nc.scalar.activation(
                            gtan[:], pre[:, 1024:1024 + 2 * w], AF.Tanh)

                        isl = gsig[:, 0:2 * w].rearrange("p (b q) -> p b q", b=2)
                        fsl = gsig[:, 2 * w:4 * w].rearrange("p (b q) -> p b q", b=2)
                        osl = gsig[:, 4 * w:6 * w].rearrange("p (b q) -> p b q", b=2)
                        gsl = gtan[:].rearrange("p (b q) -> p b q", b=2)
                        cv = blk2(csl[dd], w)
                        hv = blk2(hsl[dd], w)
                        sv = blk2(hss[dd], w)
                        t1 = workp.tile([128, 2 * w], bf16, tag="t1",
                                        name=f"t1{dd}{t}")
                        tc_ = workp.tile([128, 2 * w], bf16, tag="tc",
                                         name=f"tc{dd}{t}")
                        t1v = t1[:].rearrange("p (b q) -> p b q", b=2)
                        tcv = tc_[:].rearrange("p (b q) -> p b q", b=2)
                        nc.vector.tensor_mul(t1v, isl, gsl)
                        nc.vector.tensor_mul(cv, cv, fsl)
                        nc.vector.tensor_add(cv, cv, t1v)
                        nc.scalar.activation(tcv, cv, AF.Tanh)
                        nc.vector.tensor_mul(hv, osl, tcv)
                        nc.vector.tensor_add(sv, sv, hv)

            # ---- logits^T = E'^T.T @ [hsum_f; hsum_b]
            with tc.tile_pool(name="lg", bufs=1, space="PSUM") as lgp:
                lg = lgp.tile([N_ENT, n], f32)
                rhs = [hss["f"][:, 0:n], hss["f"][:72, n:2 * n],
                       hss["b"][:, 0:n], hss["b"][:72, n:2 * n]]
                for j4 in range(4):
                    nc.tensor.matmul(lg[:, :], et_tiles[j4][:], rhs[j4],
                                     start=(j4 == 0), stop=(j4 == 3))
                out_sb = const.tile([N_ENT, n], f32)
                nc.vector.tensor_copy(out=out_sb[:], in_=lg[:, :])
                nc.sync.dma_start(out=out_d[:, :], in_=out_sb[:])

    nc.compile()
    return nc


# ---------------------------------------------------------------- entry points

def run(inputs, trace=False, trace_cores=None):
    from concourse.bass_utils import run_bass_kernel_spmd

    meta = _plan(inputs["span_len"], inputs["span_batch"],
                 inputs["span_token_idx"])
    key = (meta["n"], meta["n_t"], meta["ntok"])
    if key not in _CACHE:
        _CACHE[key] = _build(meta)
    nc = _CACHE[key]

    A_f, B_f, bias_f = _prep_dir(inputs["W_ih_f"], inputs["W_hh_f"],
                                 inputs["b_ih_f"], inputs["b_hh_f"])
    A_b, B_b, bias_b = _prep_dir(inputs["W_ih_b"], inputs["W_hh_b"],
                                 inputs["b_ih_b"], inputs["b_hh_b"])
    bias = np.concatenate([bias_f, bias_b], axis=1)
    ET = np.asarray(inputs["entity_embs"], np.float32).T.copy()
    table = np.ascontiguousarray(
        np.asarray(inputs["hidden_layers"], np.float32).reshape(BSZ * SEQ, D))
    gidx_all = _gidx(inputs, meta)

    in_maps = [dict(table=table, gidx=gidx_all[k], A_f=A_f, A_b=A_b,
                    B_f=B_f, B_b=B_b, bias=bias, ET=ET)
               for k in range(NCORES)]
    res = run_bass_kernel_spmd(nc, in_maps, list(range(NCORES)),
                               trace=trace, trace_cores=trace_cores)
    n, NS = meta["n"], meta["NS"]
    logits = np.zeros((NS, N_ENT), np.float32)
    for k in range(NCORES):
        outk = res.results[k]["out"]                              # [16, n]
        ids = meta["cores"][k]
        sel = ids < NS
        logits[ids[sel]] = outk[:, sel].T
    return logits, res


def kernel(**inputs):
    logits, _ = run(inputs, trace=False)
    return logits


# revision 12
# speedup vs baseline: 2.2078x; 1.0341x over previous
"""Trainium2 Bass kernel for nn_EntityPredictor (bidirectional span LSTM entity scorer).

Strategy (8 NeuronCores, data-parallel over spans):
  - Host: sort spans by length desc, pad with dummy spans so every core gets an
    identical length histogram (n=spans/core, schedule n_t = #spans with len>t,
    all even).  Spans are dealt round-robin to the 8 cores, so one compiled SPMD
    program serves all cores.
  - Device (per core), "transposed" layout (gate/hidden dim on partitions, spans
    on the free dim):
      1. Indirect-DMA gather of the valid tokens (t-major staircase) from the
         full hidden_layers table resident in DRAM.
      2. PE-transpose the gathered [tok, 768] tiles into xT [768, tok] (bf16).
      3. XG = W_ih'^T.T @ xT for both directions (bf16 matmuls into PSUM), then
         ACT-Identity copies PSUM -> SBUF bf16 with the per-gate bias fused.
      4. Recurrence, both directions interleaved step-by-step so PE/ACT/DVE
         pipeline across directions.  Per step and direction: PE computes
         identity-add of the XG slice + W_hh'^T.T @ h into a packed 3-bank PSUM
         tile; one merged ACT applies sigmoid to the i/f/o chunks and one the
         tanh to g; DVE updates c (fp32), h (bf16 slab, directly in matmul-rhs
         layout) and the masked time-sum of h (fp32).  Forward walks t
         ascending; backward walks t DESCENDING, which makes the x_rev reversal
         free (every span consumes token t at global step t in both directions).
      5. logits^T [16, n] = E'^T.T @ [hsum_f; hsum_b] (fp32) and DMA out.
  - Host: unpermute spans, drop dummies, return [1280, 16] float32.

Gate padding: each gate block (200) is padded to 256 so gate boundaries align
with 128-partition chunks; gate order is permuted to (i, f, o, g) so chunks 0-5
are sigmoid and chunks 6-7 tanh.  Padded rows produce h=0 and contribute nothing.
"""

import numpy as np

BSZ, SEQ, D, H, L = 64, 512, 768, 200, 5
N_ENT = 16
NCORES = 8
HP = 256          # padded gate block
G4 = 4 * HP       # 1024 padded gate rows
KT = [(0, 128), (128, 72)]  # hidden-dim k-tiles (200 rows)

_CACHE = {}


# ---------------------------------------------------------------- host planning

def _plan(span_len, span_batch, span_token_idx):
    lens = np.asarray(span_len).astype(np.int64)
    NS = lens.shape[0]
    flat = (np.asarray(span_batch).astype(np.int64)[:, None] * SEQ
            + np.asarray(span_token_idx).astype(np.int64))       # [NS, L]
    hist = np.bincount(lens, minlength=L + 1)[1:]
    # pad classes to multiples of 16 -> per-core class counts even -> even n_t
    pad = (-hist) % (2 * NCORES)
    all_len = np.concatenate([lens, np.repeat(np.arange(1, L + 1), pad)])
    order = np.argsort(-all_len, kind="stable")                  # length desc
    N = all_len.shape[0]
    n = N // NCORES
    cores = [order[k::NCORES] for k in range(NCORES)]            # [n] ids, len desc
    n_t = tuple(int((all_len[cores[0]] > t).sum()) for t in range(L))
    for k in range(1, NCORES):
        assert tuple(int((all_len[cores[k]] > t).sum()) for t in range(L)) == n_t
    assert all(w % 2 == 0 for w in n_t) and n % 2 == 0
    offs = np.concatenate([[0], np.cumsum(n_t)]).astype(int)
    ntok = int(offs[-1])
    nchunk = (ntok + 127) // 128
    ntok_pad = nchunk * 128
    return dict(cores=cores, n=n, n_t=n_t, offs=offs, ntok=ntok,
                nchunk=nchunk, ntok_pad=ntok_pad, NS=NS,
                n_pad_spans=int(pad.sum()))


def _gidx(inputs, meta):
    flat = (np.asarray(inputs["span_batch"]).astype(np.int64)[:, None] * SEQ
            + np.asarray(inputs["span_token_idx"]).astype(np.int64))
    all_flat = np.concatenate(
        [flat, np.zeros((meta["n_pad_spans"], L), np.int64)])
    n_t, ntok_pad, nchunk = meta["n_t"], meta["ntok_pad"], meta["nchunk"]
    out = np.zeros((NCORES, 128, nchunk), np.int32)
    for k in range(NCORES):
        ids = meta["cores"][k]
        lst = np.concatenate([all_flat[ids[: n_t[t]], t] for t in range(L)])
        buf = np.zeros(ntok_pad, np.int64)
        buf[: meta["ntok"]] = lst
        out[k] = buf.reshape(nchunk, 128).T.astype(np.int32)
    return out


def _prep_dir(W_ih, W_hh, b_ih, b_hh):
    """Pad gates 200->256, permute gate order (i,f,g,o)->(i,f,o,g), transpose."""
    import ml_dtypes
    W_ih = np.asarray(W_ih, np.float32)
    W_hh = np.asarray(W_hh, np.float32)
    b = np.asarray(b_ih, np.float32) + np.asarray(b_hh, np.float32)
    A = np.zeros((G4, D), np.float32)
    B = np.zeros((G4, H), np.float32)
    bias = np.zeros((G4,), np.float32)
    for newg, oldg in enumerate([0, 1, 3, 2]):                   # i, f, o, g
        dst = slice(newg * HP, newg * HP + H)
        src = slice(oldg * H, (oldg + 1) * H)
        A[dst] = W_ih[src]
        B[dst] = W_hh[src]
        bias[dst] = b[src]
    # A^T [768, 1024] bf16, B^T [200, 1024] bf16, bias [128, 8] f32
    return (A.T.astype(ml_dtypes.bfloat16).copy(),
            B.T.astype(ml_dtypes.bfloat16).copy(),
            bias.reshape(8, 128).T.copy())


# ---------------------------------------------------------------- device program

def _build(meta):
    import concourse.bacc as bacc
    import concourse.bass as bass
    import concourse.mybir as mybir
    import concourse.tile as tile
    from concourse.masks import make_identity

    f32 = mybir.dt.float32
    bf16 = mybir.dt.bfloat16
    i32 = mybir.dt.int32
    AF = mybir.ActivationFunctionType
    n, n_t, offs, ntok = meta["n"], meta["n_t"], meta["offs"], meta["ntok"]
    nchunk, ntok_pad = meta["nchunk"], meta["ntok_pad"]

    nc = bacc.Bacc("TRN2", target_bir_lowering=False, debug=False,
                   num_devices=NCORES)
    table = nc.dram_tensor("table", [BSZ * SEQ, D], bf16, kind="ExternalInput").ap()
    gidx_d = nc.dram_tensor("gidx", [128, nchunk], i32, kind="ExternalInput").ap()
    A_d = {dd: nc.dram_tensor(f"A_{dd}", [D, G4], bf16, kind="ExternalInput").ap()
           for dd in "fb"}
    B_d = {dd: nc.dram_tensor(f"B_{dd}", [H, G4], bf16, kind="ExternalInput").ap()
           for dd in "fb"}
    bias_d = nc.dram_tensor("bias", [128, 16], f32, kind="ExternalInput").ap()
    et_d = nc.dram_tensor("ET", [2 * H, N_ENT], f32, kind="ExternalInput").ap()
    out_d = nc.dram_tensor("out", [N_ENT, n], f32, kind="ExternalOutput").ap()
    idf_d = nc.dram_tensor("identf", [128, 128], f32, kind="ExternalInput").ap()
    idb_d = nc.dram_tensor("identb", [128, 128], bf16, kind="ExternalInput").ap()

    with tile.TileContext(nc) as tc:
        with tc.tile_pool(name="const", bufs=1) as const, \
             tc.tile_pool(name="gx", bufs=3) as gxp, \
             tc.tile_pool(name="state", bufs=1) as statep, \
             tc.tile_pool(name="work", bufs=2) as workp:

            idx_sb = const.tile([128, nchunk], i32)
            nc.sync.dma_start(out=idx_sb[:], in_=gidx_d[:, :])
            ident = const.tile([128, 128], f32)
            nc.sync.dma_start(out=ident[:], in_=idf_d[:, :])
            identb = const.tile([128, 128], bf16)
            nc.sync.dma_start(out=identb[:], in_=idb_d[:, :])
            bias_sb = const.tile([128, 16], f32)
            nc.sync.dma_start(out=bias_sb[:], in_=bias_d[:, :])
            et_tiles = []
            for p0, pl in [(0, 128), (128, 72), (200, 128), (328, 72)]:
                tt = const.tile([pl, N_ENT], f32, tag=f"et{p0}")
                nc.sync.dma_start(out=tt[:], in_=et_d[p0:p0 + pl, :])
                et_tiles.append(tt)
            A_sb, B_sb = {}, {}
            for dd in "fb":
                A_sb[dd] = []
                for k in range(6):
                    t_ = const.tile([128, G4], bf16, tag=f"A{dd}{k}")
                    nc.sync.dma_start(out=t_[:], in_=A_d[dd][k * 128:(k + 1) * 128, :])
                    A_sb[dd].append(t_)
                B_sb[dd] = []
                for j, (p0, pl) in enumerate(KT):
                    t_ = const.tile([pl, G4], bf16, tag=f"B{dd}{j}")
                    nc.sync.dma_start(out=t_[:], in_=B_d[dd][p0:p0 + pl, :])
                    B_sb[dd].append(t_)

            # ---- gather + transpose -> xT[k] = x^T [768, ntok_pad] bf16
            xT = [const.tile([128, ntok_pad], bf16, tag=f"xT{k}", name=f"xT{k}")
                  for k in range(6)]
            with tc.tile_pool(name="tp", bufs=2, space="PSUM") as tpp:
                for c in range(nchunk):
                    gx = gxp.tile([128, D], bf16)
                    nc.gpsimd.indirect_dma_start(
                        out=gx[:], out_offset=None, in_=table[:, :],
                        in_offset=bass.IndirectOffsetOnAxis(
                            ap=idx_sb[:, c:c + 1], axis=0))
                    for k in range(6):
                        pt = tpp.tile([128, 128], bf16)
                        nc.tensor.transpose(pt[:], gx[:, k * 128:(k + 1) * 128],
                                            identb[:])
                        nc.vector.tensor_copy(
                            out=xT[k][:, c * 128:(c + 1) * 128], in_=pt[:])

            # ---- XG for both directions -> SBUF bf16 (bias fused in the copy)
            xg_sb = {dd: [const.tile([128, ntok], bf16, tag=f"xgsb{dd}{m}",
                                     name=f"xgsb{dd}{m}")
                          for m in range(8)] for dd in "fb"}
            for di, dd in enumerate("fb"):
                with tc.tile_pool(name=f"xg{dd}", bufs=1, space="PSUM") as xgp:
                    xg = [xgp.tile([128, ntok], f32, tag=f"xg{m}", name=f"xg{dd}{m}")
                          for m in range(8)]
                    for m in range(8):
                        for k in range(6):
                            nc.tensor.matmul(
                                xg[m][:, :],
                                A_sb[dd][k][:, m * 128:(m + 1) * 128],
                                xT[k][:, :ntok],
                                start=(k == 0), stop=(k == 5))
                    for m in range(8):
                        nc.scalar.activation(
                            xg_sb[dd][m][:, :], xg[m][:, :], AF.Identity,
                            bias=bias_sb[:, di * 8 + m: di * 8 + m + 1])

            # ---- merged-direction recurrence: per global step si, forward
            # step t=si and backward step t=L-1-si share one PSUM tile; chunk m
            # holds [f-part w_f | b-part w_b] packed 2 chunks per bank.
            hsl = {}   # h slab  [128, 2n] bf16   (block j at cols [j*n, j*n+w))
            csl = {}   # c slab  [128, 2n] f32
            hss = {}   # hsum    [128, 2n] f32
            for dd in "fb":
                hsl[dd] = statep.tile([128, 2 * n], bf16, name=f"hsl{dd}")
                csl[dd] = statep.tile([128, 2 * n], f32, name=f"csl{dd}")
                hss[dd] = statep.tile([128, 2 * n], f32, name=f"hss{dd}")
                nc.vector.memset(hsl[dd][:], 0.0)
                nc.vector.memset(csl[dd][:], 0.0)
                nc.vector.memset(hss[dd][:], 0.0)

            def blk2(tile_ap, w):
                # [128, 2, w] strided view of a [128, 2n] slab (blocks at 0, n)
                return tile_ap.rearrange("p (b q) -> p b q", b=2)[:, :, :w]

            with tc.tile_pool(name="pre", bufs=1, space="PSUM") as prep:
                for si in range(L):
                    tf, tb = si, L - 1 - si
                    wf, wb = n_t[tf], n_t[tb]
                    ws = wf + wb
                    of, ob = int(offs[tf]), int(offs[tb])
                    tt = {"f": tf, "b": tb}
                    wd = {"f": wf, "b": wb}
                    od = {"f": of, "b": ob}
                    sh = {"f": 0, "b": wf}          # intra-chunk offset
                    pre = prep.tile([128, 4 * 512], f32, tag=f"pre{si % 2}",
                                    name=f"pre{si}")

                    def chunk(m, dd, w=None, ws=ws, pre=pre, sh=sh, wd=wd):
                        off = (m // 2) * 512 + (m % 2) * ws + sh[dd]
                        return pre[:, off:off + wd[dd]]

                    for dd in "fb":
                        for m in range(8):
                            nc.tensor.matmul(
                                chunk(m, dd), identb[:],
                                xg_sb[dd][m][:, od[dd]:od[dd] + wd[dd]],
                                start=(dd == "f" and m % 2 == 0), stop=False)
                    for dd in "fb":
                        for j, (p0, pl) in enumerate(KT):
                            for m in range(8):
                                nc.tensor.matmul(
                                    chunk(m, dd),
                                    B_sb[dd][j][:, m * 128:(m + 1) * 128],
                                    hsl[dd][:pl, j * n:j * n + wd[dd]],
                                    start=False,
                                    stop=(dd == "b" and j == 1 and m % 2 == 1))
                    # merged activations: chunks 0-5 sigmoid (banks 0-2), 6-7 tanh
                    gsig = workp.tile([128, 6 * ws], bf16, tag="gsig",
                                      name=f"gsig{si}")
                    gtan = workp.tile([128, 2 * ws], bf16, tag="gtan",
                                      name=f"gtan{si}")
                    pre4 = pre[:, :].rearrange("p (b q) -> p b q", b=4)
                    nc.scalar.activation(
                        gsig[:].rearrange("p (b q) -> p b q", b=3),
                        pre4[:, 0:3, 0:2 * ws], AF.Sigmoid)
                    nc.scalar.activation(
                        gtan[:], pre[:, 3 * 512:3 * 512 + 2 * ws], AF.Tanh)

                    gs6 = gsig[:].rearrange("p (c q) -> p c q", c=6)
                    gt2 = gtan[:].rearrange("p (c q) -> p c q", c=2)
                    for dd in "fb":
                        w = wd[dd]
                        lo, hi = sh[dd], sh[dd] + w
                        isl = gs6[:, 0:2, lo:hi]
                        fsl = gs6[:, 2:4, lo:hi]
                        osl = gs6[:, 4:6, lo:hi]
                        gsl = gt2[:, :, lo:hi]
                        cv = blk2(csl[dd], w)
                        hv = blk2(hsl[dd], w)
                        sv = blk2(hss[dd], w)
                        t1 = workp.tile([128, 2 * w], bf16, tag=f"t1{dd}",
                                        name=f"t1{dd}{si}")
                        tc_ = workp.tile([128, 2 * w], bf16, tag=f"tc{dd}",
                                         name=f"tc{dd}{si}")
                        t1v = t1[:].rearrange("p (b q) -> p b q", b=2)
                        tcv = tc_[:].rearrange("p (b q) -> p b q", b=2)
                        nc.vector.tensor_mul(t1v, isl, gsl)
                        nc.vector.tensor_mul(cv, cv, fsl)
                        nc.vector.tensor_add(cv, cv, t1v)
                        nc.scalar.activation(tcv, cv, AF.Tanh)
                        nc.vector.tensor_mul(hv, osl, tcv)
                        nc.vector.tensor_add(sv, sv, hv)

            # ---- logits^T = E'^T.T @ [hsum_f; hsum_b]
            with tc.tile_pool(name="lg", bufs=1, space="PSUM") as lgp:
                lg = lgp.tile([N_ENT, n], f32)
                rhs = [hss["f"][:, 0:n], hss["f"][:72, n:2 * n],
                       hss["b"][:, 0:n], hss["b"][:72, n:2 * n]]
                for j4 in range(4):
                    nc.tensor.matmul(lg[:, :], et_tiles[j4][:], rhs[j4],
                                     start=(j4 == 0), stop=(j4 == 3))
                out_sb = const.tile([N_ENT, n], f32)
                nc.vector.tensor_copy(out=out_sb[:], in_=lg[:, :])
                nc.sync.dma_start(out=out_d[:, :], in_=out_sb[:])

    nc.compile()
    return nc


# ---------------------------------------------------------------- entry points

def run(inputs, trace=False, trace_cores=None):
    from concourse.bass_utils import run_bass_kernel_spmd

    meta = _plan(inputs["span_len"], inputs["span_batch"],
                 inputs["span_token_idx"])
    key = (meta["n"], meta["n_t"], meta["ntok"])
    if key not in _CACHE:
        _CACHE[key] = _build(meta)
    nc = _CACHE[key]

    A_f, B_f, bias_f = _prep_dir(inputs["W_ih_f"], inputs["W_hh_f"],
                                 inputs["b_ih_f"], inputs["b_hh_f"])
    A_b, B_b, bias_b = _prep_dir(inputs["W_ih_b"], inputs["W_hh_b"],
                                 inputs["b_ih_b"], inputs["b_hh_b"])
    bias = np.concatenate([bias_f, bias_b], axis=1)
    ET = np.asarray(inputs["entity_embs"], np.float32).T.copy()
    import ml_dtypes
    table = np.ascontiguousarray(
        np.asarray(inputs["hidden_layers"], np.float32)
        .reshape(BSZ * SEQ, D).astype(ml_dtypes.bfloat16))
    gidx_all = _gidx(inputs, meta)
    identf = np.eye(128, dtype=np.float32)
    identb = np.eye(128).astype(ml_dtypes.bfloat16)

    in_maps = [dict(table=table, gidx=gidx_all[k], A_f=A_f, A_b=A_b,
                    B_f=B_f, B_b=B_b, bias=bias, ET=ET,
                    identf=identf, identb=identb)
               for k in range(NCORES)]
    res = run_bass_kernel_spmd(nc, in_maps, list(range(NCORES)),
                               trace=trace, trace_cores=trace_cores)
    n, NS = meta["n"], meta["NS"]
    logits = np.zeros((NS, N_ENT), np.float32)
    for k in range(NCORES):
        outk = res.results[k]["out"]                              # [16, n]
        ids = meta["cores"][k]
        sel = ids < NS
        logits[ids[sel]] = outk[:, sel].T
    return logits, res


def kernel(**inputs):
    logits, _ = run(inputs, trace=False)
    return logits
